# revision 1
# baseline (speedup 1.0000x reference)
"""Trainium2 Bass kernel for nn_DeepProbLogAdditionReasoner (Plan B).

probs[b,k] = sum_{i+j=k} p1[b,i] p2[b,j], normalized per row
(linear convolution of two length-10 vectors -> 19 bins, / total).

DFT-19 lane algorithm (28 real bilinear lanes, see kernel.py docstring).
Precision/performance split, validated against the harness gate
(rel err with denominator |expected|+1e-5, threshold 2e-2):

  - Inputs cast to bf16 on HOST: pure input perturbation -> error stays
    RELATIVE through the (all-nonneg) conv: gate ~8.5e-3.  Halves input
    DMA and makes the PE transposes 1 cycle/row instead of 2.
  - Eval matmuls in fp16, TWO passes with exactly-split weights
    A = A_hi + A_lo (each fp16): 23-bit effective weight precision,
    accumulated in fp32 PSUM.  2 cyc/row vs fp32's 4.  The moving data
    (transposed p, bf16 values) is scaled by 2^14 during the PSUM->SBUF
    cast to fp16 so no value hits the fp16 subnormal range; the scale
    cancels in the normalization.
  - z = e1*e2 and the interp matmul (z.T @ G) stay fp32: lane-value
    perturbations are amplified ~1e4x by interpolation cancellation
    (measured), so the z/G path needs >=22 mantissa bits.

Per-core layout (131072 rows): supertile = 1536 rows as [128 part,
12 chunks x 10 digits]; PE transpose -> digit-major [120, 128].
Engine split per quad (4 supertiles): PE transposes+evals+interp;
Pool: p-cast-copies + 2 z-muls; Act: e2s copies; DVE: 1 z-mul +
reciprocal + normalize muls.
"""

import os

import numpy as np
import ml_dtypes

import concourse.bass as bass
import concourse.bacc as bacc
import concourse.tile as tile
from concourse import mybir
from concourse.bass_utils import run_bass_kernel_spmd

N_CORES = 8
B_FULL = 1048576
ROWS = B_FULL // N_CORES        # 131072 rows per core
C_FULL = 12                     # chunks per supertile
ST_ROWS = 128 * C_FULL          # 1536
N_FULL_ST = ROWS // ST_ROWS     # 85 full supertiles
REM_ROWS = ROWS - N_FULL_ST * ST_ROWS   # 512
C_PART = REM_ROWS // 128        # 4
NLANE = 28
GPS = 4                         # groups per lane-set
NSET = 3
LB = int(os.environ.get("K_LB", "4"))   # supertiles per load/store DMA batch
REPEAT = int(os.environ.get("K_REPEAT", "1"))
BUFS_PTPS = int(os.environ.get("K_PTPS", "1"))
BUFS_EPS = int(os.environ.get("K_EPS", "2"))
BUFS_UPS = int(os.environ.get("K_UPS", "2"))
BUFS_PTSB = int(os.environ.get("K_PTSB", "3"))
BUFS_ZSB = int(os.environ.get("K_ZSB", "4"))
WARM = int(os.environ.get("K_WARM", "3"))
CASCADE = int(os.environ.get("K_CASCADE", "0"))
PART_ILV = int(os.environ.get("K_PARTILV", "1"))
STORE_Q = os.environ.get("K_STOREQ", "sp")
TAPER = int(os.environ.get("K_TAPER", "0"))
LOAD2_Q = os.environ.get("K_LOAD2Q", "sp")
OUT_BF16 = int(os.environ.get("K_OUTBF", "1"))
ODT_NP = None  # set below
PART_FIRST = int(os.environ.get("K_PARTFIRST", "0"))

F32 = mybir.dt.float32
F16 = mybir.dt.float16
BF16 = mybir.dt.bfloat16
ODT = BF16 if OUT_BF16 else F32
PSCALE = 16384.0                # 2^14: keeps scaled p out of fp16 subnormals


def _build_consts():
    n = 19
    i = np.arange(10)
    C = np.cos(2 * np.pi * np.outer(i, np.arange(10)) / n)
    S = np.sin(2 * np.pi * np.outer(i, np.arange(10)) / n)
    alpha, lane_desc = [C[:, 0]], [("m1", 0)]
    for t in range(1, 10):
        alpha += [C[:, t], S[:, t], C[:, t] + S[:, t]]
        lane_desc += [("m1", t), ("m2", t), ("m3", t)]
    A = np.stack(alpha, axis=1)                      # [10, 28]

    k = np.arange(n)
    G = np.zeros((NLANE, 20))
    for l, (m, t) in enumerate(lane_desc):
        ck_cos = np.cos(2 * np.pi * k * t / n) / n
        ck_sin = np.sin(2 * np.pi * k * t / n) / n
        mult = 1.0 if t == 0 else 2.0
        if m == "m1":
            G[l, :19] += mult * (ck_cos - ck_sin)
        elif m == "m2":
            G[l, :19] += mult * (-ck_cos - ck_sin)
        else:
            G[l, :19] += mult * ck_sin
    G[0, 19] = 1.0                                    # s selector

    evalW = np.zeros((120, NSET * 112), dtype=np.float64)
    for s in range(NSET):
        for g in range(GPS):
            r0 = 10 * (GPS * s + g)
            c0 = 112 * s + NLANE * g
            evalW[r0:r0 + 10, c0:c0 + NLANE] = A
    interpW = np.zeros((112, 80), dtype=np.float64)
    for g in range(GPS):
        interpW[NLANE * g:NLANE * g + NLANE, 20 * g:20 * g + 20] = G

    eW_hi = evalW.astype(np.float16)
    eW_lo = (evalW - eW_hi.astype(np.float64)).astype(np.float16)
    return {
        "evalW_hi": eW_hi,
        "evalW_lo": eW_lo,
        "interpW": interpW.astype(np.float32),
    }


def _emit_unit(nc, pools, cst, p1L, p2L, oL, st_idx, n_st):
    """Emit ops for n_st (1..4) full supertiles ("quad" unit).

    p1L/p2L: bf16 load tiles [128, LB*120]; oL: out tile [128, LB*228];
    st_idx: index of the first supertile within the batch.
    """
    (pTps, pTsb, eps, zsb, ups, rsb) = pools
    w = n_st * 128          # transposed-cols in this unit

    p1t_ps = pTps.tile([120, 512], BF16, tag="p1t_ps")
    p2t_ps = pTps.tile([120, 512], BF16, tag="p2t_ps")
    for h in range(n_st):
        sl = slice(120 * (st_idx + h), 120 * (st_idx + h) + 120)
        nc.tensor.transpose(p1t_ps[:, 128 * h:128 * (h + 1)], p1L[:, sl], cst["ident"][:])
        nc.tensor.transpose(p2t_ps[:, 128 * h:128 * (h + 1)], p2L[:, sl], cst["ident"][:])
    # PSUM bf16 -> SBUF fp16, scaled by 2^14 (Act: out = Copy(in*scale))
    p1t = pTsb.tile([120, 512], F16, tag="p1t")
    p2t = pTsb.tile([120, 512], F16, tag="p2t")
    nc.scalar.activation(p1t[:, :w], p1t_ps[:, :w],
                         mybir.ActivationFunctionType.Copy, scale=PSCALE)
    nc.scalar.activation(p2t[:, :w], p2t_ps[:, :w],
                         mybir.ActivationFunctionType.Copy, scale=PSCALE)

    # two outU PSUM tiles: supertiles (0,1) -> A, (2,3) -> B
    n_a = min(n_st, 2)
    n_b = n_st - n_a
    outU_a = ups.tile([128, 480], F32, tag="outU")
    outU_b = ups.tile([128, 480], F32, tag="outU", name="outU_b") if n_b else None
    for s in range(NSET):
        e1 = eps.tile([112, 512], F32, tag="e1")
        e2 = eps.tile([112, 512], F32, tag="e2")
        wsl = slice(112 * s, 112 * (s + 1))
        # 2-pass fp16 eval: e = A_hi.T@p + A_lo.T@p accumulated in PSUM.
        # e2 first so its Act staging copy overlaps the e1 matmuls
        nc.tensor.matmul(e2[:, :w], cst["evalW_hi"][:, wsl], p2t[:, :w],
                         start=True, stop=False)
        nc.tensor.matmul(e2[:, :w], cst["evalW_lo"][:, wsl], p2t[:, :w],
                         start=False, stop=True)
        nc.tensor.matmul(e1[:, :w], cst["evalW_hi"][:, wsl], p1t[:, :w],
                         start=True, stop=False)
        nc.tensor.matmul(e1[:, :w], cst["evalW_lo"][:, wsl], p1t[:, :w],
                         start=False, stop=True)
        # TensorTensor may read at most one PSUM operand -> stage e2 in SBUF
        e2s = zsb.tile([112, 512], F32, tag="e2s")
        nc.scalar.copy(e2s[:, :w], e2[:, :w])
        z = zsb.tile([112, 512], F32, tag="z")
        nc.vector.tensor_mul(z[:, :w], e1[:, :w], e2s[:, :w])
        for h in range(n_st):
            dst = outU_a if h < 2 else outU_b
            nc.tensor.matmul(dst[:, 240 * (h % 2) + 80 * s:240 * (h % 2) + 80 * (s + 1)],
                             z[:, 128 * h:128 * (h + 1)], cst["interpW"][:])

    for dst, nh, st0 in ((outU_a, n_a, st_idx), (outU_b, n_b, st_idx + 2)):
        if not nh:
            continue
        nuk = nh * C_FULL
        outU_v = dst[:].rearrange("p (c k) -> p c k", k=20)[:, :nuk, :]
        r = rsb.tile([128, 24], F32, tag="r")
        nc.vector.reciprocal(r[:, :nuk], outU_v[:, :, 19])
        o_v = oL[:, 228 * st0:228 * (st0 + nh)].rearrange(
            "p (c k) -> p c k", k=19)
        r_b = r[:, :nuk].unsqueeze(2).broadcast_to([128, nuk, 19])
        nc.vector.tensor_mul(o_v, outU_v[:, :, 0:19], r_b)


def _emit_partial(nc, pools, cst, p1P, p2P, oP):
    """Remainder: C_PART chunks = 512 rows.  p1P/p2P [128, 40] bf16."""
    (pTps, pTsb, eps, zsb, ups, rsb) = pools
    cp = C_PART
    pt_p = 10 * cp
    p1t_ps = pTps.tile([120, 512], BF16, tag="p1t_ps", name="p1t_ps_part")
    p2t_ps = pTps.tile([120, 512], BF16, tag="p2t_ps", name="p2t_ps_part")
    nc.tensor.transpose(p1t_ps[0:pt_p, 0:128], p1P, cst["ident"][:])
    nc.tensor.transpose(p2t_ps[0:pt_p, 0:128], p2P, cst["ident"][:])
    p1t_t = pTsb.tile([120, 512], F16, tag="p1t", name="p1t_part")
    p2t_t = pTsb.tile([120, 512], F16, tag="p2t", name="p2t_part")
    p1t = p1t_t[0:pt_p, 0:128]
    p2t = p2t_t[0:pt_p, 0:128]
    nc.scalar.activation(p1t, p1t_ps[0:pt_p, 0:128],
                         mybir.ActivationFunctionType.Copy, scale=PSCALE)
    nc.scalar.activation(p2t, p2t_ps[0:pt_p, 0:128],
                         mybir.ActivationFunctionType.Copy, scale=PSCALE)

    e1 = eps.tile([NLANE * cp, 128], F32, tag="e1")
    e2 = eps.tile([NLANE * cp, 128], F32, tag="e2")
    nc.tensor.matmul(e1[:], cst["evalW_hi"][0:10 * cp, 0:NLANE * cp], p1t,
                     start=True, stop=False)
    nc.tensor.matmul(e1[:], cst["evalW_lo"][0:10 * cp, 0:NLANE * cp], p1t,
                     start=False, stop=True)
    nc.tensor.matmul(e2[:], cst["evalW_hi"][0:10 * cp, 0:NLANE * cp], p2t,
                     start=True, stop=False)
    nc.tensor.matmul(e2[:], cst["evalW_lo"][0:10 * cp, 0:NLANE * cp], p2t,
                     start=False, stop=True)
    e2s = zsb.tile([NLANE * cp, 128], F32, tag="e2s")
    nc.scalar.copy(e2s[:], e2[:])
    z = zsb.tile([NLANE * cp, 128], F32, tag="z")
    nc.vector.tensor_mul(z[:], e1[:], e2s[:])
    outU = ups.tile([128, 20 * cp], F32, tag="outU")
    nc.tensor.matmul(outU[:], z[:], cst["interpW"][0:NLANE * cp, 0:20 * cp])

    outU_v = outU[:].rearrange("p (c k) -> p c k", k=20)
    r = rsb.tile([128, 24], F32, tag="r")
    nc.vector.reciprocal(r[:, :cp], outU_v[:, :, 19])
    o_v = oP.rearrange("p (c k) -> p c k", k=19)
    r_b = r[:, :cp].unsqueeze(2).broadcast_to([128, cp, 19])
    nc.vector.tensor_mul(o_v, outU_v[:, :, 0:19], r_b)


def _build_program():
    nc = bacc.Bacc("TRN2", target_bir_lowering=False, debug=False,
                   enable_asserts=False, num_devices=N_CORES)
    p1_d = nc.dram_tensor("p1", [ROWS, 10], BF16, kind="ExternalInput").ap()
    p2_d = nc.dram_tensor("p2", [ROWS, 10], BF16, kind="ExternalInput").ap()
    evalW_hi_d = nc.dram_tensor("evalW_hi", [120, NSET * 112], F16,
                                kind="ExternalInput").ap()
    evalW_lo_d = nc.dram_tensor("evalW_lo", [120, NSET * 112], F16,
                                kind="ExternalInput").ap()
    interpW_d = nc.dram_tensor("interpW", [112, 80], F32, kind="ExternalInput").ap()
    out_d = nc.dram_tensor("out", [ROWS, 19], ODT, kind="ExternalOutput").ap()

    STORE_ENG = nc.scalar if STORE_Q == "act" else nc.sync
    LOAD2_ENG = nc.scalar if LOAD2_Q == "act" else nc.sync
    with tile.TileContext(nc) as tc:
        with (
            tc.tile_pool(name="const", bufs=1) as constp,
            tc.tile_pool(name="load", bufs=int(os.environ.get("K_LOADP", "2"))) as loadp,
            tc.tile_pool(name="pTps", bufs=BUFS_PTPS, space="PSUM") as pTps,
            tc.tile_pool(name="pTsb", bufs=BUFS_PTSB) as pTsb,
            tc.tile_pool(name="eps", bufs=BUFS_EPS, space="PSUM") as eps,
            tc.tile_pool(name="zsb", bufs=BUFS_ZSB) as zsb,
            tc.tile_pool(name="ups", bufs=BUFS_UPS, space="PSUM") as ups,
            tc.tile_pool(name="rsb", bufs=2) as rsb,
            tc.tile_pool(name="outp", bufs=int(os.environ.get("K_OUTP", "2"))) as outp,
        ):
            cst = {}
            # bf16 identity built on-chip (transposes take bf16 operands)
            identi = constp.tile([128, 128], mybir.dt.int32, tag="identi",
                                 name="ident_i")
            nc.gpsimd.iota(identi[:], pattern=[[1, 128]], base=0,
                           channel_multiplier=-1)
            cst["ident"] = constp.tile([128, 128], BF16, tag="ident", name="ident_t")
            nc.vector.tensor_scalar(cst["ident"][:], identi[:], 0, None,
                                    mybir.AluOpType.is_equal)
            cst["evalW_hi"] = constp.tile([120, NSET * 112], F16, tag="evalW_hi",
                                          name="evalW_hi_t")
            nc.sync.dma_start(cst["evalW_hi"][:], evalW_hi_d)
            cst["evalW_lo"] = constp.tile([120, NSET * 112], F16, tag="evalW_lo",
                                          name="evalW_lo_t")
            nc.sync.dma_start(cst["evalW_lo"][:], evalW_lo_d)
            cst["interpW"] = constp.tile([112, 80], F32, tag="interpW",
                                         name="interpW_t")
            nc.sync.dma_start(cst["interpW"][:], interpW_d)
            pools = (pTps, pTsb, eps, zsb, ups, rsb)

            # Prologue: make PE observe each constant's readiness once
            # (matmult instructions accept only ONE sync wait).
            scratch_bf = pTps.tile([120, 512], BF16, tag="p1t_ps",
                                   name="scratch_bf")  # warm-up target
            nc.tensor.transpose(scratch_bf[0:120, 0:128],
                                cst["ident"][:, 0:120], cst["ident"][:])
            scratch = ups.tile([128, 480], F32, tag="outU", name="scratch_ps")
            for wname in ("evalW_hi", "evalW_lo", "interpW"):
                wD = cst[wname]
                mm = min(112, wD.shape[1])
                nc.tensor.matmul(scratch[0:mm, 0:1], wD[:, 0:mm],
                                 wD[0:wD.shape[0], 0:1])

            # partial supertile first: its long dependency chain overlaps
            # with the main stream instead of serializing at the end
            if REM_ROWS and PART_FIRST:
                rows = slice(N_FULL_ST * ST_ROWS, ROWS)
                p1P = loadp.tile([128, 40], BF16, tag="p1P")
                p2P = loadp.tile([128, 40], BF16, tag="p2P")
                for pP, p_d in ((p1P, p1_d), (p2P, p2_d)):
                    nc.sync.dma_start(
                        pP[:], p_d[rows, :].rearrange("(p c) i -> p (c i)", p=128))
                oP = outp.tile([128, C_PART * 19], ODT, tag="oP")
                _emit_partial(nc, pools, cst, p1P[:], p2P[:], oP[:])
                STORE_ENG.dma_start(
                    out_d[rows, :].rearrange("(p c) k -> p (c k)", p=128), oP[:])

            # full-supertile batches (REPEAT>1 only for wall-clock timing)
            # first batch is small so the PE pipeline starts after a short DMA
            for _rep in range(REPEAT):
                s0 = 0
                batch_i = 0
                while s0 < N_FULL_ST:
                    if WARM and batch_i == 0:
                        nb = WARM
                    elif CASCADE and WARM and batch_i == 1:
                        nb = min(2 * WARM, LB)
                    else:
                        nb = LB
                    nb = min(nb, N_FULL_ST - s0)
                    if TAPER and N_FULL_ST - s0 - nb == 0 and nb > TAPER:
                        nb -= TAPER   # split off a small final batch
                    is_last = (s0 + nb == N_FULL_ST) and (_rep == REPEAT - 1)
                    rows = slice(ST_ROWS * s0, ST_ROWS * (s0 + nb))
                    p1L = loadp.tile([128, LB * 120], BF16, tag="p1L")
                    p2L = loadp.tile([128, LB * 120], BF16, tag="p2L")
                    for pL, p_d, eng in ((p1L, p1_d, nc.sync),
                                         (p2L, p2_d, LOAD2_ENG)):
                        # row = base + nb*12*p + 12*s + c: one contiguous
                        # DRAM run per partition
                        eng.dma_start(
                            pL[:].rearrange("p (s x) -> p s x",
                                            x=120)[:, :nb, :],
                            p_d[rows, :].rearrange(
                                "(p s c) i -> p s (c i)", p=128, c=C_FULL))
                    if is_last and REM_ROWS and not PART_FIRST and PART_ILV:
                        # interleave the 512-row remainder with the last
                        # batch so its chain overlaps instead of tailing
                        prows = slice(N_FULL_ST * ST_ROWS, ROWS)
                        p1P = loadp.tile([128, 40], BF16, tag="p1P")
                        p2P = loadp.tile([128, 40], BF16, tag="p2P")
                        for pP, p_d in ((p1P, p1_d), (p2P, p2_d)):
                            nc.sync.dma_start(
                                pP[:], p_d[prows, :].rearrange(
                                    "(p c) i -> p (c i)", p=128))
                    oL = outp.tile([128, LB * 228], ODT, tag="oL")
                    st = 0
                    while st < nb:
                        n_st = min(4, nb - st)
                        _emit_unit(nc, pools, cst, p1L, p2L, oL, st, n_st)
                        if is_last and REM_ROWS and not PART_FIRST and PART_ILV and st == 0:
                            oP = outp.tile([128, C_PART * 19], ODT, tag="oP")
                            _emit_partial(nc, pools, cst, p1P[:], p2P[:], oP[:])
                            STORE_ENG.dma_start(
                                out_d[prows, :].rearrange(
                                    "(p c) k -> p (c k)", p=128), oP[:])
                        st += n_st
                    STORE_ENG.dma_start(
                        out_d[rows, :].rearrange("(p s c) k -> p s (c k)", p=128, c=C_FULL),
                        oL[:].rearrange("p (s x) -> p s x", x=228)[:, :nb, :])
                    s0 += nb
                    batch_i += 1
            if REM_ROWS and not PART_FIRST and not PART_ILV:
                rows = slice(N_FULL_ST * ST_ROWS, ROWS)
                p1P = loadp.tile([128, 40], BF16, tag="p1P")
                p2P = loadp.tile([128, 40], BF16, tag="p2P")
                for pP, p_d in ((p1P, p1_d), (p2P, p2_d)):
                    nc.sync.dma_start(
                        pP[:], p_d[rows, :].rearrange("(p c) i -> p (c i)", p=128))
                oP = outp.tile([128, C_PART * 19], ODT, tag="oP")
                _emit_partial(nc, pools, cst, p1P[:], p2P[:], oP[:])
                STORE_ENG.dma_start(
                    out_d[rows, :].rearrange("(p c) k -> p (c k)", p=128), oP[:])

    nc.compile()
    return nc


def _run(p1, p2, trace=False, trace_kwargs=None):
    p1 = np.asarray(p1)
    p2 = np.asarray(p2)
    assert p1.shape == (B_FULL, 10) and p2.shape == (B_FULL, 10)
    p1b = np.ascontiguousarray(p1.astype(ml_dtypes.bfloat16))
    p2b = np.ascontiguousarray(p2.astype(ml_dtypes.bfloat16))
    cs = _build_consts()
    nc = _build_program()
    p1s = p1b.reshape(N_CORES, ROWS, 10)
    p2s = p2b.reshape(N_CORES, ROWS, 10)
    in_maps = [
        {"p1": p1s[c], "p2": p2s[c], "evalW_hi": cs["evalW_hi"],
         "evalW_lo": cs["evalW_lo"], "interpW": cs["interpW"]}
        for c in range(N_CORES)
    ]
    res = run_bass_kernel_spmd(
        nc, in_maps, core_ids=list(range(N_CORES)),
        trace=trace, **(trace_kwargs or {}))
    out = np.concatenate([res.results[c]["out"] for c in range(N_CORES)], axis=0)
    return np.asarray(out, dtype=np.float32).reshape(B_FULL, 19), res


def kernel(p1, p2):
    out, _ = _run(p1, p2, trace=False)
    return out



# revision 4
# speedup vs baseline: 1.0665x; 1.0665x over previous
"""Trainium2 Bass kernel for nn_DeepProbLogAdditionReasoner (Plan B).

probs[b,k] = sum_{i+j=k} p1[b,i] p2[b,j], normalized per row
(linear convolution of two length-10 vectors -> 19 bins, / total).

DFT-19 lane algorithm (28 real bilinear lanes, see kernel.py docstring).
Precision/performance split, validated against the harness gate
(rel err with denominator |expected|+1e-5, threshold 2e-2):

  - Inputs cast to bf16 on HOST: pure input perturbation -> error stays
    RELATIVE through the (all-nonneg) conv: gate ~8.5e-3.  Halves input
    DMA and makes the PE transposes 1 cycle/row instead of 2.
  - Eval matmuls in fp16, TWO passes with exactly-split weights
    A = A_hi + A_lo (each fp16): 23-bit effective weight precision,
    accumulated in fp32 PSUM.  2 cyc/row vs fp32's 4.  The moving data
    (transposed p, bf16 values) is scaled by 2^14 during the PSUM->SBUF
    cast to fp16 so no value hits the fp16 subnormal range; the scale
    cancels in the normalization.
  - z = e1*e2 and the interp matmul (z.T @ G) stay fp32: lane-value
    perturbations are amplified ~1e4x by interpolation cancellation
    (measured), so the z/G path needs >=22 mantissa bits.

Per-core layout (131072 rows): supertile = 1536 rows as [128 part,
12 chunks x 10 digits]; PE transpose -> digit-major [120, 128].
Engine split per quad (4 supertiles): PE transposes+evals+interp;
Pool: p-cast-copies + 2 z-muls; Act: e2s copies; DVE: 1 z-mul +
reciprocal + normalize muls.
"""

import os

import numpy as np
import ml_dtypes

import concourse.bass as bass
import concourse.bacc as bacc
import concourse.tile as tile
from concourse import mybir
from concourse.bass_utils import run_bass_kernel_spmd

N_CORES = 8
B_FULL = 1048576
ROWS = B_FULL // N_CORES        # 131072 rows per core
C_FULL = 12                     # chunks per supertile
ST_ROWS = 128 * C_FULL          # 1536
N_FULL_ST = ROWS // ST_ROWS     # 85 full supertiles
REM_ROWS = ROWS - N_FULL_ST * ST_ROWS   # 512
C_PART = REM_ROWS // 128        # 4
NLANE = 28
GPS = 4                         # groups per lane-set
NSET = 3
LB = int(os.environ.get("K_LB", "4"))   # supertiles per load/store DMA batch
REPEAT = int(os.environ.get("K_REPEAT", "1"))
BUFS_PTPS = int(os.environ.get("K_PTPS", "1"))
BUFS_EPS = int(os.environ.get("K_EPS", "2"))
BUFS_UPS = int(os.environ.get("K_UPS", "2"))
BUFS_PTSB = int(os.environ.get("K_PTSB", "3"))
BUFS_ZSB = int(os.environ.get("K_ZSB", "4"))
WARM = int(os.environ.get("K_WARM", "3"))
CASCADE = int(os.environ.get("K_CASCADE", "0"))
PART_ILV = int(os.environ.get("K_PARTILV", "1"))
STORE_Q = os.environ.get("K_STOREQ", "sp")
TAPER = int(os.environ.get("K_TAPER", "0"))
LOAD2_Q = os.environ.get("K_LOAD2Q", "sp")
OUT_BF16 = int(os.environ.get("K_OUTBF", "1"))
ODT_NP = None  # set below
PART_FIRST = int(os.environ.get("K_PARTFIRST", "0"))

F32 = mybir.dt.float32
F16 = mybir.dt.float16
BF16 = mybir.dt.bfloat16
ODT = BF16 if OUT_BF16 else F32
# K_PDT: dtype of the transposed/moving p data ("f16" w/ 2^14 scale, or "bf16")
# K_WSPLIT: eval-weight split scheme ("f16x2" = fp16 hi+lo, "bf16x3" = bf16 3-way)
PDT_NAME = os.environ.get("K_PDT", "f16")
WSPLIT = os.environ.get("K_WSPLIT", "f16x2")
PDT = F16 if PDT_NAME == "f16" else BF16
EVAL_WDT = F16 if WSPLIT.startswith("f16") else BF16
N_EVAL_PASS = int(WSPLIT[-1])
PSCALE = 16384.0 if PDT_NAME == "f16" else 1.0  # 2^14 keeps scaled p out of fp16 subnormals


def _build_consts():
    n = 19
    i = np.arange(10)
    C = np.cos(2 * np.pi * np.outer(i, np.arange(10)) / n)
    S = np.sin(2 * np.pi * np.outer(i, np.arange(10)) / n)
    alpha, lane_desc = [C[:, 0]], [("m1", 0)]
    for t in range(1, 10):
        alpha += [C[:, t], S[:, t], C[:, t] + S[:, t]]
        lane_desc += [("m1", t), ("m2", t), ("m3", t)]
    A = np.stack(alpha, axis=1)                      # [10, 28]

    k = np.arange(n)
    G = np.zeros((NLANE, 20))
    for l, (m, t) in enumerate(lane_desc):
        ck_cos = np.cos(2 * np.pi * k * t / n) / n
        ck_sin = np.sin(2 * np.pi * k * t / n) / n
        mult = 1.0 if t == 0 else 2.0
        if m == "m1":
            G[l, :19] += mult * (ck_cos - ck_sin)
        elif m == "m2":
            G[l, :19] += mult * (-ck_cos - ck_sin)
        else:
            G[l, :19] += mult * ck_sin
    G[0, 19] = 1.0                                    # s selector

    evalW = np.zeros((120, NSET * 112), dtype=np.float64)
    for s in range(NSET):
        for g in range(GPS):
            r0 = 10 * (GPS * s + g)
            c0 = 112 * s + NLANE * g
            evalW[r0:r0 + 10, c0:c0 + NLANE] = A
    interpW = np.zeros((112, 80), dtype=np.float64)
    for g in range(GPS):
        interpW[NLANE * g:NLANE * g + NLANE, 20 * g:20 * g + 20] = G

    wnp = np.float16 if EVAL_WDT == F16 else ml_dtypes.bfloat16
    parts, resid = [], evalW.copy()
    for _ in range(N_EVAL_PASS):
        p = resid.astype(wnp)
        parts.append(p)
        resid = resid - p.astype(np.float64)
    out = {f"evalW_{i}": parts[i] for i in range(N_EVAL_PASS)}
    out["interpW"] = interpW.astype(np.float32)
    return out


def _emit_unit(nc, pools, cst, p1L, p2L, oL, st_idx, n_st):
    """Emit ops for n_st (1..4) full supertiles ("quad" unit).

    p1L/p2L: bf16 load tiles [128, LB*120]; oL: out tile [128, LB*228];
    st_idx: index of the first supertile within the batch.
    """
    (pTps, pTsb, eps, zsb, ups, rsb) = pools
    w = n_st * 128          # transposed-cols in this unit

    p1t_ps = pTps.tile([120, 512], BF16, tag="p1t_ps")
    p2t_ps = pTps.tile([120, 512], BF16, tag="p2t_ps")
    for h in range(n_st):
        sl = slice(120 * (st_idx + h), 120 * (st_idx + h) + 120)
        nc.tensor.transpose(p1t_ps[:, 128 * h:128 * (h + 1)], p1L[:, sl], cst["ident"][:])
        nc.tensor.transpose(p2t_ps[:, 128 * h:128 * (h + 1)], p2L[:, sl], cst["ident"][:])
    # PSUM bf16 -> SBUF fp16, scaled by 2^14 (Act: out = Copy(in*scale))
    p1t = pTsb.tile([120, 512], F16, tag="p1t")
    p2t = pTsb.tile([120, 512], F16, tag="p2t")
    nc.scalar.activation(p1t[:, :w], p1t_ps[:, :w],
                         mybir.ActivationFunctionType.Copy, scale=PSCALE)
    nc.scalar.activation(p2t[:, :w], p2t_ps[:, :w],
                         mybir.ActivationFunctionType.Copy, scale=PSCALE)

    # two outU PSUM tiles: supertiles (0,1) -> A, (2,3) -> B
    n_a = min(n_st, 2)
    n_b = n_st - n_a
    outU_a = ups.tile([128, 480], F32, tag="outU")
    outU_b = ups.tile([128, 480], F32, tag="outU", name="outU_b") if n_b else None
    for s in range(NSET):
        e1 = eps.tile([112, 512], F32, tag="e1")
        e2 = eps.tile([112, 512], F32, tag="e2")
        wsl = slice(112 * s, 112 * (s + 1))
        # 2-pass fp16 eval: e = A_hi.T@p + A_lo.T@p accumulated in PSUM.
        # e2 first so its Act staging copy overlaps the e1 matmuls
        nc.tensor.matmul(e2[:, :w], cst["evalW_hi"][:, wsl], p2t[:, :w],
                         start=True, stop=False)
        nc.tensor.matmul(e2[:, :w], cst["evalW_lo"][:, wsl], p2t[:, :w],
                         start=False, stop=True)
        nc.tensor.matmul(e1[:, :w], cst["evalW_hi"][:, wsl], p1t[:, :w],
                         start=True, stop=False)
        nc.tensor.matmul(e1[:, :w], cst["evalW_lo"][:, wsl], p1t[:, :w],
                         start=False, stop=True)
        # TensorTensor may read at most one PSUM operand -> stage e2 in SBUF
        e2s = zsb.tile([112, 512], F32, tag="e2s")
        nc.scalar.copy(e2s[:, :w], e2[:, :w])
        z = zsb.tile([112, 512], F32, tag="z")
        nc.vector.tensor_mul(z[:, :w], e1[:, :w], e2s[:, :w])
        for h in range(n_st):
            dst = outU_a if h < 2 else outU_b
            nc.tensor.matmul(dst[:, 240 * (h % 2) + 80 * s:240 * (h % 2) + 80 * (s + 1)],
                             z[:, 128 * h:128 * (h + 1)], cst["interpW"][:])

    for dst, nh, st0 in ((outU_a, n_a, st_idx), (outU_b, n_b, st_idx + 2)):
        if not nh:
            continue
        nuk = nh * C_FULL
        outU_v = dst[:].rearrange("p (c k) -> p c k", k=20)[:, :nuk, :]
        r = rsb.tile([128, 24], F32, tag="r")
        nc.vector.reciprocal(r[:, :nuk], outU_v[:, :, 19])
        o_v = oL[:, 228 * st0:228 * (st0 + nh)].rearrange(
            "p (c k) -> p c k", k=19)
        r_b = r[:, :nuk].unsqueeze(2).broadcast_to([128, nuk, 19])
        nc.vector.tensor_mul(o_v, outU_v[:, :, 0:19], r_b)


def _emit_partial(nc, pools, cst, p1P, p2P, oP):
    """Remainder: C_PART chunks = 512 rows.  p1P/p2P [128, 40] bf16."""
    (pTps, pTsb, eps, zsb, ups, rsb) = pools
    cp = C_PART
    pt_p = 10 * cp
    p1t_ps = pTps.tile([120, 512], BF16, tag="p1t_ps", name="p1t_ps_part")
    p2t_ps = pTps.tile([120, 512], BF16, tag="p2t_ps", name="p2t_ps_part")
    nc.tensor.transpose(p1t_ps[0:pt_p, 0:128], p1P, cst["ident"][:])
    nc.tensor.transpose(p2t_ps[0:pt_p, 0:128], p2P, cst["ident"][:])
    p1t_t = pTsb.tile([120, 512], F16, tag="p1t", name="p1t_part")
    p2t_t = pTsb.tile([120, 512], F16, tag="p2t", name="p2t_part")
    p1t = p1t_t[0:pt_p, 0:128]
    p2t = p2t_t[0:pt_p, 0:128]
    nc.scalar.activation(p1t, p1t_ps[0:pt_p, 0:128],
                         mybir.ActivationFunctionType.Copy, scale=PSCALE)
    nc.scalar.activation(p2t, p2t_ps[0:pt_p, 0:128],
                         mybir.ActivationFunctionType.Copy, scale=PSCALE)

    e1 = eps.tile([NLANE * cp, 128], F32, tag="e1")
    e2 = eps.tile([NLANE * cp, 128], F32, tag="e2")
    nc.tensor.matmul(e1[:], cst["evalW_hi"][0:10 * cp, 0:NLANE * cp], p1t,
                     start=True, stop=False)
    nc.tensor.matmul(e1[:], cst["evalW_lo"][0:10 * cp, 0:NLANE * cp], p1t,
                     start=False, stop=True)
    nc.tensor.matmul(e2[:], cst["evalW_hi"][0:10 * cp, 0:NLANE * cp], p2t,
                     start=True, stop=False)
    nc.tensor.matmul(e2[:], cst["evalW_lo"][0:10 * cp, 0:NLANE * cp], p2t,
                     start=False, stop=True)
    e2s = zsb.tile([NLANE * cp, 128], F32, tag="e2s")
    nc.scalar.copy(e2s[:], e2[:])
    z = zsb.tile([NLANE * cp, 128], F32, tag="z")
    nc.vector.tensor_mul(z[:], e1[:], e2s[:])
    outU = ups.tile([128, 20 * cp], F32, tag="outU")
    nc.tensor.matmul(outU[:], z[:], cst["interpW"][0:NLANE * cp, 0:20 * cp])

    outU_v = outU[:].rearrange("p (c k) -> p c k", k=20)
    r = rsb.tile([128, 24], F32, tag="r")
    nc.vector.reciprocal(r[:, :cp], outU_v[:, :, 19])
    o_v = oP.rearrange("p (c k) -> p c k", k=19)
    r_b = r[:, :cp].unsqueeze(2).broadcast_to([128, cp, 19])
    nc.vector.tensor_mul(o_v, outU_v[:, :, 0:19], r_b)


def _build_program():
    nc = bacc.Bacc("TRN2", target_bir_lowering=False, debug=False,
                   enable_asserts=False, num_devices=N_CORES)
    p1_d = nc.dram_tensor("p1", [ROWS, 10], BF16, kind="ExternalInput").ap()
    p2_d = nc.dram_tensor("p2", [ROWS, 10], BF16, kind="ExternalInput").ap()
    evalW_hi_d = nc.dram_tensor("evalW_hi", [120, NSET * 112], F16,
                                kind="ExternalInput").ap()
    evalW_lo_d = nc.dram_tensor("evalW_lo", [120, NSET * 112], F16,
                                kind="ExternalInput").ap()
    interpW_d = nc.dram_tensor("interpW", [112, 80], F32, kind="ExternalInput").ap()
    out_d = nc.dram_tensor("out", [ROWS, 19], ODT, kind="ExternalOutput").ap()

    STORE_ENG = nc.scalar if STORE_Q == "act" else nc.sync
    LOAD2_ENG = nc.scalar if LOAD2_Q == "act" else nc.sync
    with tile.TileContext(nc) as tc:
        with (
            tc.tile_pool(name="const", bufs=1) as constp,
            tc.tile_pool(name="load", bufs=int(os.environ.get("K_LOADP", "2"))) as loadp,
            tc.tile_pool(name="pTps", bufs=BUFS_PTPS, space="PSUM") as pTps,
            tc.tile_pool(name="pTsb", bufs=BUFS_PTSB) as pTsb,
            tc.tile_pool(name="eps", bufs=BUFS_EPS, space="PSUM") as eps,
            tc.tile_pool(name="zsb", bufs=BUFS_ZSB) as zsb,
            tc.tile_pool(name="ups", bufs=BUFS_UPS, space="PSUM") as ups,
            tc.tile_pool(name="rsb", bufs=2) as rsb,
            tc.tile_pool(name="outp", bufs=int(os.environ.get("K_OUTP", "2"))) as outp,
        ):
            cst = {}
            # bf16 identity built on-chip (transposes take bf16 operands)
            identi = constp.tile([128, 128], mybir.dt.int32, tag="identi",
                                 name="ident_i")
            nc.gpsimd.iota(identi[:], pattern=[[1, 128]], base=0,
                           channel_multiplier=-1)
            cst["ident"] = constp.tile([128, 128], BF16, tag="ident", name="ident_t")
            nc.vector.tensor_scalar(cst["ident"][:], identi[:], 0, None,
                                    mybir.AluOpType.is_equal)
            cst["evalW_hi"] = constp.tile([120, NSET * 112], F16, tag="evalW_hi",
                                          name="evalW_hi_t")
            nc.sync.dma_start(cst["evalW_hi"][:], evalW_hi_d)
            cst["evalW_lo"] = constp.tile([120, NSET * 112], F16, tag="evalW_lo",
                                          name="evalW_lo_t")
            nc.sync.dma_start(cst["evalW_lo"][:], evalW_lo_d)
            cst["interpW"] = constp.tile([112, 80], F32, tag="interpW",
                                         name="interpW_t")
            nc.sync.dma_start(cst["interpW"][:], interpW_d)
            pools = (pTps, pTsb, eps, zsb, ups, rsb)

            # Prologue: make PE observe each constant's readiness once
            # (matmult instructions accept only ONE sync wait).
            scratch_bf = pTps.tile([120, 512], BF16, tag="p1t_ps",
                                   name="scratch_bf")  # warm-up target
            nc.tensor.transpose(scratch_bf[0:120, 0:128],
                                cst["ident"][:, 0:120], cst["ident"][:])
            scratch = ups.tile([128, 480], F32, tag="outU", name="scratch_ps")
            for wname in ("evalW_hi", "evalW_lo", "interpW"):
                wD = cst[wname]
                mm = min(112, wD.shape[1])
                nc.tensor.matmul(scratch[0:mm, 0:1], wD[:, 0:mm],
                                 wD[0:wD.shape[0], 0:1])

            # partial supertile first: its long dependency chain overlaps
            # with the main stream instead of serializing at the end
            if REM_ROWS and PART_FIRST:
                rows = slice(N_FULL_ST * ST_ROWS, ROWS)
                p1P = loadp.tile([128, 40], BF16, tag="p1P")
                p2P = loadp.tile([128, 40], BF16, tag="p2P")
                for pP, p_d in ((p1P, p1_d), (p2P, p2_d)):
                    nc.sync.dma_start(
                        pP[:], p_d[rows, :].rearrange("(p c) i -> p (c i)", p=128))
                oP = outp.tile([128, C_PART * 19], ODT, tag="oP")
                _emit_partial(nc, pools, cst, p1P[:], p2P[:], oP[:])
                STORE_ENG.dma_start(
                    out_d[rows, :].rearrange("(p c) k -> p (c k)", p=128), oP[:])

            # full-supertile batches (REPEAT>1 only for wall-clock timing)
            # first batch is small so the PE pipeline starts after a short DMA
            for _rep in range(REPEAT):
                s0 = 0
                batch_i = 0
                while s0 < N_FULL_ST:
                    if WARM and batch_i == 0:
                        nb = WARM
                    elif CASCADE and WARM and batch_i == 1:
                        nb = min(2 * WARM, LB)
                    else:
                        nb = LB
                    nb = min(nb, N_FULL_ST - s0)
                    if TAPER and N_FULL_ST - s0 - nb == 0 and nb > TAPER:
                        nb -= TAPER   # split off a small final batch
                    is_last = (s0 + nb == N_FULL_ST) and (_rep == REPEAT - 1)
                    rows = slice(ST_ROWS * s0, ST_ROWS * (s0 + nb))
                    p1L = loadp.tile([128, LB * 120], BF16, tag="p1L")
                    p2L = loadp.tile([128, LB * 120], BF16, tag="p2L")
                    for pL, p_d, eng in ((p1L, p1_d, nc.sync),
                                         (p2L, p2_d, LOAD2_ENG)):
                        # row = base + nb*12*p + 12*s + c: one contiguous
                        # DRAM run per partition
                        eng.dma_start(
                            pL[:].rearrange("p (s x) -> p s x",
                                            x=120)[:, :nb, :],
                            p_d[rows, :].rearrange(
                                "(p s c) i -> p s (c i)", p=128, c=C_FULL))
                    if is_last and REM_ROWS and not PART_FIRST and PART_ILV:
                        # interleave the 512-row remainder with the last
                        # batch so its chain overlaps instead of tailing
                        prows = slice(N_FULL_ST * ST_ROWS, ROWS)
                        p1P = loadp.tile([128, 40], BF16, tag="p1P")
                        p2P = loadp.tile([128, 40], BF16, tag="p2P")
                        for pP, p_d in ((p1P, p1_d), (p2P, p2_d)):
                            nc.sync.dma_start(
                                pP[:], p_d[prows, :].rearrange(
                                    "(p c) i -> p (c i)", p=128))
                    oL = outp.tile([128, LB * 228], ODT, tag="oL")
                    st = 0
                    while st < nb:
                        n_st = min(4, nb - st)
                        _emit_unit(nc, pools, cst, p1L, p2L, oL, st, n_st)
                        if is_last and REM_ROWS and not PART_FIRST and PART_ILV and st == 0:
                            oP = outp.tile([128, C_PART * 19], ODT, tag="oP")
                            _emit_partial(nc, pools, cst, p1P[:], p2P[:], oP[:])
                            STORE_ENG.dma_start(
                                out_d[prows, :].rearrange(
                                    "(p c) k -> p (c k)", p=128), oP[:])
                        st += n_st
                    STORE_ENG.dma_start(
                        out_d[rows, :].rearrange("(p s c) k -> p s (c k)", p=128, c=C_FULL),
                        oL[:].rearrange("p (s x) -> p s x", x=228)[:, :nb, :])
                    s0 += nb
                    batch_i += 1
            if REM_ROWS and not PART_FIRST and not PART_ILV:
                rows = slice(N_FULL_ST * ST_ROWS, ROWS)
                p1P = loadp.tile([128, 40], BF16, tag="p1P")
                p2P = loadp.tile([128, 40], BF16, tag="p2P")
                for pP, p_d in ((p1P, p1_d), (p2P, p2_d)):
                    nc.sync.dma_start(
                        pP[:], p_d[rows, :].rearrange("(p c) i -> p (c i)", p=128))
                oP = outp.tile([128, C_PART * 19], ODT, tag="oP")
                _emit_partial(nc, pools, cst, p1P[:], p2P[:], oP[:])
                STORE_ENG.dma_start(
                    out_d[rows, :].rearrange("(p c) k -> p (c k)", p=128), oP[:])

    nc.compile()
    return nc


def _make_in_maps(p1, p2):
    p1 = np.asarray(p1)
    p2 = np.asarray(p2)
    assert p1.shape == (B_FULL, 10) and p2.shape == (B_FULL, 10)
    p1b = np.ascontiguousarray(p1.astype(ml_dtypes.bfloat16))
    p2b = np.ascontiguousarray(p2.astype(ml_dtypes.bfloat16))
    cs = _build_consts()
    p1s = p1b.reshape(N_CORES, ROWS, 10)
    p2s = p2b.reshape(N_CORES, ROWS, 10)
    return [
        {"p1": p1s[c], "p2": p2s[c], "evalW_hi": cs["evalW_hi"],
         "evalW_lo": cs["evalW_lo"], "interpW": cs["interpW"]}
        for c in range(N_CORES)
    ]


def _run(p1, p2, trace=False, trace_kwargs=None):
    in_maps = _make_in_maps(p1, p2)
    nc = _build_program()
    res = run_bass_kernel_spmd(
        nc, in_maps, core_ids=list(range(N_CORES)),
        trace=trace, **(trace_kwargs or {}))
    out = np.concatenate([res.results[c]["out"] for c in range(N_CORES)], axis=0)
    return np.asarray(out, dtype=np.float32).reshape(B_FULL, 19), res


def kernel(p1, p2):
    out, _ = _run(p1, p2, trace=False)
    return out



# revision 17
# speedup vs baseline: 1.5120x; 1.4178x over previous
"""Trainium2 Bass kernel for nn_DeepProbLogAdditionReasoner (Plan B).

probs[b,k] = sum_{i+j=k} p1[b,i] p2[b,j], normalized per row
(linear convolution of two length-10 vectors -> 19 bins, / total).

DFT-19 lane algorithm (28 real bilinear lanes, see kernel.py docstring).
Precision/performance split, validated against the harness gate
(rel err with denominator |expected|+1e-5, threshold 2e-2):

  - Inputs cast to bf16 on HOST: pure input perturbation -> error stays
    RELATIVE through the (all-nonneg) conv: gate ~8.5e-3.  Halves input
    DMA and makes the PE transposes 1 cycle/row instead of 2.
  - Eval matmuls in fp16, TWO passes with exactly-split weights
    A = A_hi + A_lo (each fp16): 23-bit effective weight precision,
    accumulated in fp32 PSUM.  2 cyc/row vs fp32's 4.  The moving data
    (transposed p, bf16 values) is scaled by 2^14 during the PSUM->SBUF
    cast to fp16 so no value hits the fp16 subnormal range; the scale
    cancels in the normalization.
  - z = e1*e2 and the interp matmul (z.T @ G) stay fp32: lane-value
    perturbations are amplified ~1e4x by interpolation cancellation
    (measured), so the z/G path needs >=22 mantissa bits.

Per-core layout (131072 rows): supertile = 1536 rows as [128 part,
12 chunks x 10 digits]; PE transpose -> digit-major [120, 128].
Engine split per quad (4 supertiles): PE transposes+evals+interp;
Pool: p-cast-copies + 2 z-muls; Act: e2s copies; DVE: 1 z-mul +
reciprocal + normalize muls.
"""

import os

import numpy as np
import ml_dtypes

import concourse.bass as bass
import concourse.bacc as bacc
import concourse.tile as tile
from concourse import mybir
from concourse.bass_utils import run_bass_kernel_spmd

N_CORES = 8
B_FULL = 1048576
ROWS = B_FULL // N_CORES        # 131072 rows per core
C_FULL = 12                     # chunks per supertile
ST_ROWS = 128 * C_FULL          # 1536
N_FULL_ST = ROWS // ST_ROWS     # 85 full supertiles
REM_ROWS = ROWS - N_FULL_ST * ST_ROWS   # 512
C_PART = REM_ROWS // 128        # 4
NLANE = 28
GPS = 4                         # groups per lane-set
NSET = 3
LB = int(os.environ.get("K_LB", "4"))   # supertiles per load/store DMA batch
REPEAT = int(os.environ.get("K_REPEAT", "1"))
BUFS_PTPS = int(os.environ.get("K_PTPS", "1"))
BUFS_EPS = int(os.environ.get("K_EPS", "2"))
BUFS_UPS = int(os.environ.get("K_UPS", "2"))
BUFS_PTSB = int(os.environ.get("K_PTSB", "3"))
BUFS_ZSB = int(os.environ.get("K_ZSB", "4"))
WARM = int(os.environ.get("K_WARM", "3"))
CASCADE = int(os.environ.get("K_CASCADE", "0"))
PART_ILV = int(os.environ.get("K_PARTILV", "1"))
STORE_Q = os.environ.get("K_STOREQ", "sp")
TAPER = int(os.environ.get("K_TAPER", "0"))
LOAD2_Q = os.environ.get("K_LOAD2Q", "sp")
OUT_BF16 = int(os.environ.get("K_OUTBF", "1"))
ODT_NP = None  # set below
PART_FIRST = int(os.environ.get("K_PARTFIRST", "0"))

F32 = mybir.dt.float32
F16 = mybir.dt.float16
BF16 = mybir.dt.bfloat16
ODT = BF16 if OUT_BF16 else F32
# K_PDT: dtype of the transposed/moving p data ("f16" w/ 2^14 scale, or "bf16")
# K_WSPLIT: eval-weight split scheme ("f16x2" = fp16 hi+lo, "bf16x3" = bf16 3-way)
PDT_NAME = os.environ.get("K_PDT", "f16")
WSPLIT = os.environ.get("K_WSPLIT", "f16x2")
PDT = F16 if PDT_NAME == "f16" else BF16
EVAL_WDT = F16 if WSPLIT.startswith("f16") else BF16
N_EVAL_PASS = int(WSPLIT[-1])
# K_GDT: interp weight dtype ("f32" single pass, "f16x2", "bf16x3")
GSPLIT = os.environ.get("K_GDT", "f32")
if GSPLIT == "f32":
    INTERP_WDT, N_INTERP_PASS = F32, 1
else:
    INTERP_WDT = F16 if GSPLIT.startswith("f16") else BF16
    N_INTERP_PASS = int(GSPLIT[-1])
PSCALE = 16384.0 if PDT_NAME == "f16" else 1.0  # 2^14 keeps scaled p out of fp16 subnormals


def _build_consts():
    n = 19
    i = np.arange(10)
    C = np.cos(2 * np.pi * np.outer(i, np.arange(10)) / n)
    S = np.sin(2 * np.pi * np.outer(i, np.arange(10)) / n)
    alpha, lane_desc = [C[:, 0]], [("m1", 0)]
    for t in range(1, 10):
        alpha += [C[:, t], S[:, t], C[:, t] + S[:, t]]
        lane_desc += [("m1", t), ("m2", t), ("m3", t)]
    A = np.stack(alpha, axis=1)                      # [10, 28]

    k = np.arange(n)
    G = np.zeros((NLANE, 20))
    for l, (m, t) in enumerate(lane_desc):
        ck_cos = np.cos(2 * np.pi * k * t / n) / n
        ck_sin = np.sin(2 * np.pi * k * t / n) / n
        mult = 1.0 if t == 0 else 2.0
        if m == "m1":
            G[l, :19] += mult * (ck_cos - ck_sin)
        elif m == "m2":
            G[l, :19] += mult * (-ck_cos - ck_sin)
        else:
            G[l, :19] += mult * ck_sin
    G[0, 19] = 1.0                                    # s selector

    evalW = np.zeros((120, NSET * 112), dtype=np.float64)
    for s in range(NSET):
        for g in range(GPS):
            r0 = 10 * (GPS * s + g)
            c0 = 112 * s + NLANE * g
            evalW[r0:r0 + 10, c0:c0 + NLANE] = A
    interpW = np.zeros((112, 80), dtype=np.float64)
    for g in range(GPS):
        interpW[NLANE * g:NLANE * g + NLANE, 20 * g:20 * g + 20] = G

    wnp = np.float16 if EVAL_WDT == F16 else ml_dtypes.bfloat16
    parts, resid = [], evalW.copy()
    for _ in range(N_EVAL_PASS):
        p = resid.astype(wnp)
        parts.append(p)
        resid = resid - p.astype(np.float64)
    out = {f"evalW_{i}": parts[i] for i in range(N_EVAL_PASS)}
    if N_INTERP_PASS == 1:
        out["interpW_0"] = interpW.astype(np.float32)
    else:
        gnp = np.float16 if INTERP_WDT == F16 else ml_dtypes.bfloat16
        resid = interpW.copy()
        for i in range(N_INTERP_PASS):
            g = resid.astype(gnp)
            out[f"interpW_{i}"] = g
            resid = resid - g.astype(np.float64)
    return out


def _emit_unit(nc, pools, cst, p1L, p2L, oL, st_idx, n_st):
    """Emit ops for n_st (1..4) full supertiles ("quad" unit).

    p1L/p2L: bf16 load tiles [128, LB*120]; oL: out tile [128, LB*228];
    st_idx: index of the first supertile within the batch.
    """
    (pTps, pTsb, eps, zsb, ups, rsb) = pools
    w = n_st * 128          # transposed-cols in this unit

    p1t_ps = pTps.tile([120, 512], BF16, tag="p1t_ps")
    p2t_ps = pTps.tile([120, 512], BF16, tag="p2t_ps")
    for h in range(n_st):
        sl = slice(120 * (st_idx + h), 120 * (st_idx + h) + 120)
        nc.tensor.transpose(p1t_ps[:, 128 * h:128 * (h + 1)], p1L[:, sl], cst["ident"][:])
        nc.tensor.transpose(p2t_ps[:, 128 * h:128 * (h + 1)], p2L[:, sl], cst["ident"][:])
    # PSUM bf16 -> SBUF PDT, scaled by PSCALE (Act: out = Copy(in*scale))
    p1t = pTsb.tile([120, 512], PDT, tag="p1t")
    p2t = pTsb.tile([120, 512], PDT, tag="p2t")
    nc.scalar.activation(p1t[:, :w], p1t_ps[:, :w],
                         mybir.ActivationFunctionType.Copy, scale=PSCALE)
    nc.scalar.activation(p2t[:, :w], p2t_ps[:, :w],
                         mybir.ActivationFunctionType.Copy, scale=PSCALE)

    # two outU PSUM tiles: supertiles (0,1) -> A, (2,3) -> B
    n_a = min(n_st, 2)
    n_b = n_st - n_a
    outU_a = ups.tile([128, 480], F32, tag="outU")
    outU_b = ups.tile([128, 480], F32, tag="outU", name="outU_b") if n_b else None
    for s in range(NSET):
        e1 = eps.tile([112, 512], F32, tag="e1")
        e2 = eps.tile([112, 512], F32, tag="e2")
        wsl = slice(112 * s, 112 * (s + 1))
        # multi-pass eval: e = sum_p A_p.T@p accumulated in PSUM.
        # e2 first so its Act staging copy overlaps the e1 matmuls
        for p in range(N_EVAL_PASS):
            nc.tensor.matmul(e2[:, :w], cst[f"evalW_{p}"][:, wsl], p2t[:, :w],
                             start=(p == 0), stop=(p == N_EVAL_PASS - 1))
        for p in range(N_EVAL_PASS):
            nc.tensor.matmul(e1[:, :w], cst[f"evalW_{p}"][:, wsl], p1t[:, :w],
                             start=(p == 0), stop=(p == N_EVAL_PASS - 1))
        # TensorTensor may read at most one PSUM operand -> stage e2 in SBUF
        e2s = zsb.tile([112, 512], F32, tag="e2s")
        nc.scalar.copy(e2s[:, :w], e2[:, :w])
        z = zsb.tile([112, 512], F32, tag="z")
        nc.vector.tensor_mul(z[:, :w], e1[:, :w], e2s[:, :w])
        for h in range(n_st):
            dst = outU_a if h < 2 else outU_b
            for p in range(N_INTERP_PASS):
                nc.tensor.matmul(
                    dst[:, 240 * (h % 2) + 80 * s:240 * (h % 2) + 80 * (s + 1)],
                    z[:, 128 * h:128 * (h + 1)], cst[f"interpW_{p}"][:],
                    start=(p == 0), stop=(p == N_INTERP_PASS - 1))

    for dst, nh, st0 in ((outU_a, n_a, st_idx), (outU_b, n_b, st_idx + 2)):
        if not nh:
            continue
        nuk = nh * C_FULL
        outU_v = dst[:].rearrange("p (c k) -> p c k", k=20)[:, :nuk, :]
        r = rsb.tile([128, 24], F32, tag="r")
        nc.vector.reciprocal(r[:, :nuk], outU_v[:, :, 19])
        o_v = oL[:, 228 * st0:228 * (st0 + nh)].rearrange(
            "p (c k) -> p c k", k=19)
        r_b = r[:, :nuk].unsqueeze(2).broadcast_to([128, nuk, 19])
        nc.vector.tensor_mul(o_v, outU_v[:, :, 0:19], r_b)


def _emit_partial(nc, pools, cst, p1P, p2P, oP):
    """Remainder: C_PART chunks = 512 rows.  p1P/p2P [128, 40] bf16."""
    (pTps, pTsb, eps, zsb, ups, rsb) = pools
    cp = C_PART
    pt_p = 10 * cp
    p1t_ps = pTps.tile([120, 512], BF16, tag="p1t_ps", name="p1t_ps_part")
    p2t_ps = pTps.tile([120, 512], BF16, tag="p2t_ps", name="p2t_ps_part")
    nc.tensor.transpose(p1t_ps[0:pt_p, 0:128], p1P, cst["ident"][:])
    nc.tensor.transpose(p2t_ps[0:pt_p, 0:128], p2P, cst["ident"][:])
    p1t_t = pTsb.tile([120, 512], PDT, tag="p1t", name="p1t_part")
    p2t_t = pTsb.tile([120, 512], PDT, tag="p2t", name="p2t_part")
    p1t = p1t_t[0:pt_p, 0:128]
    p2t = p2t_t[0:pt_p, 0:128]
    nc.scalar.activation(p1t, p1t_ps[0:pt_p, 0:128],
                         mybir.ActivationFunctionType.Copy, scale=PSCALE)
    nc.scalar.activation(p2t, p2t_ps[0:pt_p, 0:128],
                         mybir.ActivationFunctionType.Copy, scale=PSCALE)

    e1 = eps.tile([NLANE * cp, 128], F32, tag="e1")
    e2 = eps.tile([NLANE * cp, 128], F32, tag="e2")
    for p in range(N_EVAL_PASS):
        nc.tensor.matmul(e1[:], cst[f"evalW_{p}"][0:10 * cp, 0:NLANE * cp], p1t,
                         start=(p == 0), stop=(p == N_EVAL_PASS - 1))
    for p in range(N_EVAL_PASS):
        nc.tensor.matmul(e2[:], cst[f"evalW_{p}"][0:10 * cp, 0:NLANE * cp], p2t,
                         start=(p == 0), stop=(p == N_EVAL_PASS - 1))
    e2s = zsb.tile([NLANE * cp, 128], F32, tag="e2s")
    nc.scalar.copy(e2s[:], e2[:])
    z = zsb.tile([NLANE * cp, 128], F32, tag="z")
    nc.vector.tensor_mul(z[:], e1[:], e2s[:])
    outU = ups.tile([128, 20 * cp], F32, tag="outU")
    for p in range(N_INTERP_PASS):
        nc.tensor.matmul(outU[:], z[:], cst[f"interpW_{p}"][0:NLANE * cp, 0:20 * cp],
                         start=(p == 0), stop=(p == N_INTERP_PASS - 1))

    outU_v = outU[:].rearrange("p (c k) -> p c k", k=20)
    r = rsb.tile([128, 24], F32, tag="r")
    nc.vector.reciprocal(r[:, :cp], outU_v[:, :, 19])
    o_v = oP.rearrange("p (c k) -> p c k", k=19)
    r_b = r[:, :cp].unsqueeze(2).broadcast_to([128, cp, 19])
    nc.vector.tensor_mul(o_v, outU_v[:, :, 0:19], r_b)


def _build_program():
    nc = bacc.Bacc("TRN2", target_bir_lowering=False, debug=False,
                   enable_asserts=False, num_devices=N_CORES)
    p1_d = nc.dram_tensor("p1", [ROWS, 10], BF16, kind="ExternalInput").ap()
    p2_d = nc.dram_tensor("p2", [ROWS, 10], BF16, kind="ExternalInput").ap()
    evalW_d = [
        nc.dram_tensor(f"evalW_{p}", [120, NSET * 112], EVAL_WDT,
                       kind="ExternalInput").ap()
        for p in range(N_EVAL_PASS)
    ]
    interpW_d = [
        nc.dram_tensor(f"interpW_{p}", [112, 80], INTERP_WDT,
                       kind="ExternalInput").ap()
        for p in range(N_INTERP_PASS)
    ]
    out_d = nc.dram_tensor("out", [ROWS, 19], ODT, kind="ExternalOutput").ap()

    STORE_ENG = nc.scalar if STORE_Q == "act" else nc.sync
    LOAD2_ENG = nc.scalar if LOAD2_Q == "act" else nc.sync
    with tile.TileContext(nc) as tc:
        with (
            tc.tile_pool(name="const", bufs=1) as constp,
            tc.tile_pool(name="load", bufs=int(os.environ.get("K_LOADP", "2"))) as loadp,
            tc.tile_pool(name="pTps", bufs=BUFS_PTPS, space="PSUM") as pTps,
            tc.tile_pool(name="pTsb", bufs=BUFS_PTSB) as pTsb,
            tc.tile_pool(name="eps", bufs=BUFS_EPS, space="PSUM") as eps,
            tc.tile_pool(name="zsb", bufs=BUFS_ZSB) as zsb,
            tc.tile_pool(name="ups", bufs=BUFS_UPS, space="PSUM") as ups,
            tc.tile_pool(name="rsb", bufs=2) as rsb,
            tc.tile_pool(name="outp", bufs=int(os.environ.get("K_OUTP", "2"))) as outp,
        ):
            cst = {}
            # bf16 identity built on-chip (transposes take bf16 operands)
            identi = constp.tile([128, 128], mybir.dt.int32, tag="identi",
                                 name="ident_i")
            nc.gpsimd.iota(identi[:], pattern=[[1, 128]], base=0,
                           channel_multiplier=-1)
            cst["ident"] = constp.tile([128, 128], BF16, tag="ident", name="ident_t")
            nc.vector.tensor_scalar(cst["ident"][:], identi[:], 0, None,
                                    mybir.AluOpType.is_equal)
            for p in range(N_EVAL_PASS):
                cst[f"evalW_{p}"] = constp.tile(
                    [120, NSET * 112], EVAL_WDT, tag=f"evalW_{p}",
                    name=f"evalW_{p}_t")
                nc.sync.dma_start(cst[f"evalW_{p}"][:], evalW_d[p])
            for p in range(N_INTERP_PASS):
                cst[f"interpW_{p}"] = constp.tile(
                    [112, 80], INTERP_WDT, tag=f"interpW_{p}",
                    name=f"interpW_{p}_t")
                nc.sync.dma_start(cst[f"interpW_{p}"][:], interpW_d[p])
            pools = (pTps, pTsb, eps, zsb, ups, rsb)

            # Prologue: make PE observe each constant's readiness once
            # (matmult instructions accept only ONE sync wait).
            scratch_bf = pTps.tile([120, 512], BF16, tag="p1t_ps",
                                   name="scratch_bf")  # warm-up target
            nc.tensor.transpose(scratch_bf[0:120, 0:128],
                                cst["ident"][:, 0:120], cst["ident"][:])
            scratch = ups.tile([128, 480], F32, tag="outU", name="scratch_ps")
            for wname in ([f"evalW_{p}" for p in range(N_EVAL_PASS)]
                          + [f"interpW_{p}" for p in range(N_INTERP_PASS)]):
                wD = cst[wname]
                mm = min(112, wD.shape[1])
                nc.tensor.matmul(scratch[0:mm, 0:1], wD[:, 0:mm],
                                 wD[0:wD.shape[0], 0:1])

            # partial supertile first: its long dependency chain overlaps
            # with the main stream instead of serializing at the end
            if REM_ROWS and PART_FIRST:
                rows = slice(N_FULL_ST * ST_ROWS, ROWS)
                p1P = loadp.tile([128, 40], BF16, tag="p1P")
                p2P = loadp.tile([128, 40], BF16, tag="p2P")
                for pP, p_d in ((p1P, p1_d), (p2P, p2_d)):
                    nc.sync.dma_start(
                        pP[:], p_d[rows, :].rearrange("(p c) i -> p (c i)", p=128))
                oP = outp.tile([128, C_PART * 19], ODT, tag="oP")
                _emit_partial(nc, pools, cst, p1P[:], p2P[:], oP[:])
                STORE_ENG.dma_start(
                    out_d[rows, :].rearrange("(p c) k -> p (c k)", p=128), oP[:])

            # full-supertile batches (REPEAT>1 only for wall-clock timing)
            # first batch is small so the PE pipeline starts after a short DMA
            for _rep in range(REPEAT):
                s0 = 0
                batch_i = 0
                while s0 < N_FULL_ST:
                    if WARM and batch_i == 0:
                        nb = WARM
                    elif CASCADE and WARM and batch_i == 1:
                        nb = min(2 * WARM, LB)
                    else:
                        nb = LB
                    nb = min(nb, N_FULL_ST - s0)
                    if TAPER and N_FULL_ST - s0 - nb == 0 and nb > TAPER:
                        nb -= TAPER   # split off a small final batch
                    is_last = (s0 + nb == N_FULL_ST) and (_rep == REPEAT - 1)
                    rows = slice(ST_ROWS * s0, ST_ROWS * (s0 + nb))
                    p1L = loadp.tile([128, LB * 120], BF16, tag="p1L")
                    p2L = loadp.tile([128, LB * 120], BF16, tag="p2L")
                    for pL, p_d, eng in ((p1L, p1_d, nc.sync),
                                         (p2L, p2_d, LOAD2_ENG)):
                        # row = base + nb*12*p + 12*s + c: one contiguous
                        # DRAM run per partition
                        eng.dma_start(
                            pL[:].rearrange("p (s x) -> p s x",
                                            x=120)[:, :nb, :],
                            p_d[rows, :].rearrange(
                                "(p s c) i -> p s (c i)", p=128, c=C_FULL))
                    if is_last and REM_ROWS and not PART_FIRST and PART_ILV:
                        # interleave the 512-row remainder with the last
                        # batch so its chain overlaps instead of tailing
                        prows = slice(N_FULL_ST * ST_ROWS, ROWS)
                        p1P = loadp.tile([128, 40], BF16, tag="p1P")
                        p2P = loadp.tile([128, 40], BF16, tag="p2P")
                        for pP, p_d in ((p1P, p1_d), (p2P, p2_d)):
                            nc.sync.dma_start(
                                pP[:], p_d[prows, :].rearrange(
                                    "(p c) i -> p (c i)", p=128))
                    oL = outp.tile([128, LB * 228], ODT, tag="oL")
                    st = 0
                    while st < nb:
                        n_st = min(4, nb - st)
                        _emit_unit(nc, pools, cst, p1L, p2L, oL, st, n_st)
                        if is_last and REM_ROWS and not PART_FIRST and PART_ILV and st == 0:
                            oP = outp.tile([128, C_PART * 19], ODT, tag="oP")
                            _emit_partial(nc, pools, cst, p1P[:], p2P[:], oP[:])
                            STORE_ENG.dma_start(
                                out_d[prows, :].rearrange(
                                    "(p c) k -> p (c k)", p=128), oP[:])
                        st += n_st
                    STORE_ENG.dma_start(
                        out_d[rows, :].rearrange("(p s c) k -> p s (c k)", p=128, c=C_FULL),
                        oL[:].rearrange("p (s x) -> p s x", x=228)[:, :nb, :])
                    s0 += nb
                    batch_i += 1
            if REM_ROWS and not PART_FIRST and not PART_ILV:
                rows = slice(N_FULL_ST * ST_ROWS, ROWS)
                p1P = loadp.tile([128, 40], BF16, tag="p1P")
                p2P = loadp.tile([128, 40], BF16, tag="p2P")
                for pP, p_d in ((p1P, p1_d), (p2P, p2_d)):
                    nc.sync.dma_start(
                        pP[:], p_d[rows, :].rearrange("(p c) i -> p (c i)", p=128))
                oP = outp.tile([128, C_PART * 19], ODT, tag="oP")
                _emit_partial(nc, pools, cst, p1P[:], p2P[:], oP[:])
                STORE_ENG.dma_start(
                    out_d[rows, :].rearrange("(p c) k -> p (c k)", p=128), oP[:])

    nc.compile()
    return nc


def _make_in_maps(p1, p2):
    p1 = np.asarray(p1)
    p2 = np.asarray(p2)
    assert p1.shape == (B_FULL, 10) and p2.shape == (B_FULL, 10)
    p1b = np.ascontiguousarray(p1.astype(ml_dtypes.bfloat16))
    p2b = np.ascontiguousarray(p2.astype(ml_dtypes.bfloat16))
    cs = _build_consts()
    p1s = p1b.reshape(N_CORES, ROWS, 10)
    p2s = p2b.reshape(N_CORES, ROWS, 10)
    return [
        {"p1": p1s[c], "p2": p2s[c], **cs}
        for c in range(N_CORES)
    ]


def _run(p1, p2, trace=False, trace_kwargs=None):
    in_maps = _make_in_maps(p1, p2)
    nc = _build_program()
    res = run_bass_kernel_spmd(
        nc, in_maps, core_ids=list(range(N_CORES)),
        trace=trace, **(trace_kwargs or {}))
    out = np.concatenate([res.results[c]["out"] for c in range(N_CORES)], axis=0)
    return np.asarray(out, dtype=np.float32).reshape(B_FULL, 19), res


def kernel(p1, p2):
    out, _ = _run(p1, p2, trace=False)
    return out

